# revision 1
# baseline (speedup 1.0000x reference)
"""TRN2 Bass kernel for soft 2D polygon rasterization (1024x1024, 64-edge polygon).

Strategy (one SPMD program on 8 cores, per-core behavior fully data-driven):
  - Layout: x (columns) on partitions, y (rows) on the free axis. The image is
    split into 64 tiles of [128 cols x 128 rows]; each core processes 8,
    assigned by a host-side load-balancing local search that minimizes the
    padded per-phase slot maxima (all cores run the same instruction stream).
  - Inside/outside parity: host builds a per-column histogram of edge-crossing
    rows with alternating +1/-1 weights (sorted order), so a prefix sum along y
    gives parity (0/1) directly. The prefix sum is one f32 matmul per tile
    against a triangular 0/1 matrix on the otherwise-idle TensorEngine. The
    bbox+threshold band mask is folded in as +-131072 histogram step entries
    (y) and per-column offsets (x), driving sd2 below the -450 zero cutoff for
    out-of-band pixels (which are provably >= 30 px from the boundary).
  - Distance: sigmoid(+-d2) is within e^-30 ~ 9e-14 of exact 1.0/0.0 once
    d2 >= 30, far below the scale-relative absmax gate, so only pixels within
    ~5.5 px of the boundary need the true distance. The host culls, per tile,
    the edges/vertices within reach. Per edge:
    d2_seg >= max(BIG*overshoot, c^2) with equality wherever it matters;
    c^2 and BIG*|overshoot| are single fused ACT ops (func(scale*y + bias[p]))
    -- every 3rd slot computes c^2 on the vector engine instead to balance the
    engines -- combined with one scalar_tensor_tensor (the first slot writes
    d2 directly) and one tensor_tensor min. Vertices: one ACT Square plus one
    fused add-min scalar_tensor_tensor.
  - Finals: sd2 = (parity-0.5)*d2min via one STT per tile; two tiles share a
    [128,256] buffer so one ACT Sigmoid(2*sd2) serves both (the ACT spline
    saturates to exactly 0.0/1.0 at the extremes, so no explicit far-field
    zero test is needed); DMA out. Host reassembles 64 tiles, transposes.
"""
import os
import numpy as np

W = H = 1024
NCORES = 8
OCT_H = 128          # tile rows
NOCT = 8             # tiles per core
SIGMA = 1.0
THRESHOLD = 30.0
BIG = 1e6
R_KEEP = 4.0         # cull radius: d2>=30 saturates to within e^-30 of 0/1,
                     # far below the scale-relative absmax gate

LAST_RESULTS = None  # BassKernelResults of the most recent run (for test harness)


# ---------------------------------------------------------------------------
# host-side geometry prep
# ---------------------------------------------------------------------------

def _host_prep(polygon):
    poly = np.asarray(polygon, dtype=np.float32)
    E = poly.shape[0]
    a = poly
    b = np.roll(poly, -1, axis=0)
    ab = b - a

    # bbox band (exact f32 replication of the reference)
    x_lo = np.float32(np.floor(poly[:, 0].min()))
    y_lo = np.float32(np.floor(poly[:, 1].min()))
    x_hi = np.float32(np.floor(poly[:, 0].max()) + np.float32(1.0))
    y_hi = np.float32(np.floor(poly[:, 1].max()) + np.float32(1.0))
    thr = np.float32(THRESHOLD)
    xband_lo = x_lo - thr
    xband_hi = x_hi + thr
    yband_lo = y_lo - thr
    yband_hi = y_hi + thr

    # ---- signed crossing histogram (exact f32 semantics) ----
    PX = np.arange(W, dtype=np.float32)[None, :]
    a0 = a[:, 0:1]; a1 = a[:, 1:2]; b0 = b[:, 0:1]
    ab0 = ab[:, 0:1]; ab1 = ab[:, 1:2]
    crosses = (a0 <= PX) != (b0 <= PX)                       # [E, W]
    safe_dx = np.where(ab0 == np.float32(0.0), np.float32(1.0), ab0)
    with np.errstate(over='ignore', invalid='ignore'):
        yint = a1 + (PX - a0) * ab1 / safe_dx                # [E, W] f32
    bins = np.where(crosses, np.ceil(yint.astype(np.float64)), np.inf)
    bins = np.where(bins < 0, 0.0, bins)                     # clamp below
    bins = np.where(bins > H - 1, np.inf, bins)              # >1023 never hits
    srt = np.sort(bins, axis=0)                              # per column asc
    sign = np.where((np.arange(E)[:, None] % 2) == 0, 1.0, -1.0)
    hist = np.zeros((H, W), dtype=np.float32)
    valid = np.isfinite(srt)
    kk = srt[valid].astype(np.int64)
    jj = np.broadcast_to(np.arange(W)[None, :], (E, W))[valid]
    np.add.at(hist, (kk, jj), np.broadcast_to(sign, (E, W))[valid])
    # parity below row r0: number of bins < r0 mod 2 == signed prefix (0/1)
    csum = np.cumsum(hist, axis=0)                           # parity at row i

    # ---- per-(edge, strip) reach culling (f64 geometry) ----
    A = a.astype(np.float64); B = b.astype(np.float64); AB = B - A
    L2 = AB[:, 0] ** 2 + AB[:, 1] ** 2
    Lc = np.sqrt(np.maximum(L2, 1e-12))
    good = L2 > 1e-9

    # per octant (strip s, oct o): lists of edge ids and vertex ids
    NO = H // OCT_H
    oct_edges = [[[] for _ in range(NO)] for _ in range(8)]
    oct_verts = [[[] for _ in range(NO)] for _ in range(8)]
    for s in range(8):
        xr0, xr1 = s * 128, s * 128 + 127
        for e in range(E):
            ax, ay = A[e]; bx, by = B[e]
            if good[e]:
                lo, hi = min(ax, bx), max(ax, bx)
                if not (hi < xr0 - R_KEEP or lo > xr1 + R_KEEP):
                    ts = [0.0, 1.0]
                    if abs(bx - ax) > 1e-12:
                        for xc in (xr0 - R_KEEP, xr1 + R_KEEP):
                            t = (xc - ax) / (bx - ax)
                            if 0.0 < t < 1.0:
                                ts.append(t)
                    ts = [t for t in ts
                          if xr0 - R_KEEP - 1e-9 <= ax + (bx - ax) * t <= xr1 + R_KEEP + 1e-9]
                    if ts:
                        ys = [ay + (by - ay) * t for t in ts]
                        ylo = max(0, int(np.floor(min(ys) - R_KEEP)))
                        yhi = min(H - 1, int(np.ceil(max(ys) + R_KEEP)))
                        if ylo <= yhi:
                            for o in range(NO):
                                if max(ylo, o * OCT_H) <= min(yhi, o * OCT_H + OCT_H - 1):
                                    oct_edges[s][o].append(e)
            if xr0 - R_KEEP <= ax <= xr1 + R_KEEP:
                ylo = max(0, int(np.floor(ay - R_KEEP)))
                yhi = min(H - 1, int(np.ceil(ay + R_KEEP)))
                for o in range(NO):
                    if max(ylo, o * OCT_H) <= min(yhi, o * OCT_H + OCT_H - 1):
                        oct_verts[s][o].append(e)

    # ---- octant -> (core, phase) assignment ----
    # The SPMD program pads each phase to the max (edge, vertex) slot counts over
    # cores, so the objective is sum_k (cE*maxE_k + cV*maxV_k) after sorting
    # each core's octants by cost. LPT start + pairwise-swap local search.
    octs = [(s, o) for s in range(8) for o in range(NO)]
    nE = {so: len(oct_edges[so[0]][so[1]]) for so in octs}
    nV = {so: len(oct_verts[so[0]][so[1]]) for so in octs}
    cE, cV = 2.0, 1.0
    cost = {so: cE * nE[so] + cV * nV[so] for so in octs}

    def padded_cost(assign):
        tot = 0.0
        ranked = [sorted(a, key=lambda so: -cost[so]) for a in assign]
        for k in range(NOCT):
            tot += cE * max(nE[r[k]] for r in ranked)
            tot += cV * max(nV[r[k]] for r in ranked)
        return tot

    order = sorted(octs, key=lambda so: -cost[so])
    core_load = [0.0] * NCORES
    assign = [[] for _ in range(NCORES)]
    for so in order:
        cands = [c for c in range(NCORES) if len(assign[c]) < NOCT]
        c = min(cands, key=lambda c: core_load[c])
        assign[c].append(so)
        core_load[c] += cost[so]
    best = padded_cost(assign)
    rng = np.random.default_rng(0)
    for _ in range(8000):
        c1, c2 = rng.integers(0, NCORES, 2)
        if c1 == c2:
            continue
        i1, i2 = rng.integers(0, NOCT, 2)
        assign[c1][i1], assign[c2][i2] = assign[c2][i2], assign[c1][i1]
        newc = padded_cost(assign)
        if newc <= best:
            best = newc
        else:
            assign[c1][i1], assign[c2][i2] = assign[c2][i2], assign[c1][i1]
    core_octs = [sorted(a, key=lambda so: -cost[so]) for a in assign]

    S = [max(len(oct_edges[core_octs[c][k][0]][core_octs[c][k][1]])
             for c in range(NCORES)) for k in range(NOCT)]
    V = [max(len(oct_verts[core_octs[c][k][0]][core_octs[c][k][1]])
             for c in range(NCORES)) for k in range(NOCT)]

    # ---- per-core input tensors ----
    # coef layout per phase k: [scC, bC, scM, bM, bigl2] * S[k] then [bV, kx]*V[k]
    # then [sc05]
    # Band masking is folded into the parity matmul: out-of-band rows/columns
    # get a -BANDK offset in par (via extra histogram step entries for y, via
    # sc05 for x), which drives sd2 below the -450 zero-test. Out-of-band
    # pixels are >=30 px from the polygon so their computed d2 >= ~104 and
    # BANDK*d2 is always large enough.
    BANDK = 131072.0
    ncol = sum(5 * S[k] + 2 * V[k] + 1 for k in range(NOCT))
    xs_all = np.arange(W, dtype=np.float64)
    # first/last in-band rows (integer pixel coords, f32-exact values)
    r_lo = int(np.ceil(float(yband_lo)))
    r_hi = int(np.floor(float(yband_hi)))
    in_maps = []
    for c in range(NCORES):
        coef = np.zeros((128, ncol), dtype=np.float32)
        histc = np.zeros((NOCT, OCT_H, 128), dtype=np.float32)
        col = 0
        for k in range(NOCT):
            s, o = core_octs[c][k]
            i0 = o * OCT_H
            xs = xs_all[s * 128:(s + 1) * 128]
            elist = oct_edges[s][o]
            vlist = oct_verts[s][o]
            for si in range(S[k]):
                if si < len(elist):
                    e = elist[si]
                    L = Lc[e]
                    scC = -AB[e, 0] / L
                    bC = ((xs - A[e, 0]) * AB[e, 1] + A[e, 1] * AB[e, 0]) / L + scC * i0
                    scM = BIG * AB[e, 1] / L
                    bM = (BIG * (((xs - A[e, 0]) * AB[e, 0] - A[e, 1] * AB[e, 1]) / L
                                 - L / 2.0) + scM * i0)
                    bigl2 = BIG * L / 2.0
                else:  # dummy: candidate = 4000 everywhere (saturated, bounded)
                    scC = 0.0; bC = np.full(128, 60.0); scM = 0.0
                    bM = np.full(128, 4000.0); bigl2 = 0.0
                coef[:, col + 0] = scC
                coef[:, col + 1] = bC
                coef[:, col + 2] = scM
                coef[:, col + 3] = bM
                coef[:, col + 4] = bigl2
                col += 5
            for vi in range(V[k]):
                if vi < len(vlist):
                    e = vlist[vi]
                    coef[:, col + 0] = i0 - A[e, 1]
                    coef[:, col + 1] = np.square(xs - A[e, 0])
                else:
                    coef[:, col + 0] = 200.0   # sqv >= 4e4: never the min
                    coef[:, col + 1] = 0.0
                col += 2
            base = np.mod(csum[i0 - 1, s * 128:(s + 1) * 128], 2.0) if i0 > 0 \
                else np.zeros(128)
            # y-band step entries (local rows), plus constant part
            hloc = np.ascontiguousarray(hist[i0:i0 + OCT_H, s * 128:(s + 1) * 128])
            base_const = -BANDK
            rl, rh1 = r_lo - i0, r_hi - i0 + 1
            if rl <= 0:
                base_const += BANDK
            elif rl <= OCT_H - 1:
                hloc[rl, :] += BANDK
            if rh1 <= 0:
                base_const -= BANDK
            elif rh1 <= OCT_H - 1:
                hloc[rh1, :] -= BANDK
            xsf = xs.astype(np.float32)
            xg = np.where((xsf >= xband_lo) & (xsf <= xband_hi), 0.0, -BANDK)
            coef[:, col + 0] = 0.5 - base - base_const - xg
            col += 1
            histc[k] = hloc
        in_maps.append({
            "coef": coef,
            "hist": histc.reshape(NOCT * OCT_H, 128),
        })
    return in_maps, core_octs, S, V, ncol


# ---------------------------------------------------------------------------
# device program
# ---------------------------------------------------------------------------

def _build_program(S, V, ncol):
    import concourse.bacc as bacc
    import concourse.mybir as mybir
    from concourse.tile import TileContext

    F32 = mybir.dt.float32
    I32 = mybir.dt.int32
    BF16 = mybir.dt.bfloat16
    AF = mybir.ActivationFunctionType
    OP = mybir.AluOpType

    nc = bacc.Bacc()
    coef_in = nc.declare_dram_parameter("coef", [128, ncol], F32, isOutput=False)
    hist_in = nc.declare_dram_parameter("hist", [NOCT * OCT_H, 128], F32, isOutput=False)
    out_dram = nc.declare_dram_parameter("out", [NOCT, 128, OCT_H], F32, isOutput=True)

    with TileContext(nc) as tc:
        with tc.tile_pool(name="const", bufs=1) as cpool, \
             tc.tile_pool(name="work", bufs=8) as wpool, \
             tc.tile_pool(name="acc", bufs=4) as apool, \
             tc.tile_pool(name="ps", bufs=4, space="PSUM") as psum:

            # per-phase coef slices so phase 0 can start as soon as possible
            coef = cpool.tile([128, ncol], F32)
            cc = 0
            for k in range(NOCT):
                w = 5 * S[k] + 2 * V[k] + 1
                nc.sync.dma_start(out=coef[:, cc:cc + w], in_=coef_in[:, cc:cc + w])
                cc += w

            # warmup: trigger the ACT table load (sigmoid_and_others covers
            # Square/Abs/Sigmoid) while input DMAs are in flight
            warm = cpool.tile([128, 1], F32)
            nc.vector.memset(warm[:], 0.0)
            nc.scalar.activation(warm[:], warm[:], AF.Sigmoid, bias=0.0, scale=1.0)

            # Yr = iota f32 (row index within octant)
            yi = cpool.tile([128, OCT_H], I32)
            nc.gpsimd.iota(yi[:], pattern=[[1, OCT_H]], base=0, channel_multiplier=0)
            yr = cpool.tile([128, OCT_H], F32)
            nc.vector.tensor_copy(out=yr[:], in_=yi[:])

            # U triangular [128, 128] f32: U[kk, ii] = (kk <= ii)
            ui = cpool.tile([128, OCT_H], I32)
            nc.gpsimd.iota(ui[:], pattern=[[1, OCT_H]], base=0,
                           channel_multiplier=-1)
            ubf = cpool.tile([128, OCT_H], F32)
            nc.vector.tensor_scalar(out=ubf[:], in0=ui[:], scalar1=0, scalar2=None,
                                    op0=OP.is_ge)

            col = 0
            for k in range(NOCT):
                # parity prefix-sum matmuls
                hk0 = wpool.tile([128, 128], F32, tag="hist0")
                nc.sync.dma_start(out=hk0[:],
                                  in_=hist_in[k * OCT_H:(k + 1) * OCT_H, :])
                par = psum.tile([128, OCT_H], F32, tag="par")
                nc.tensor.matmul(par[:], lhsT=hk0[:], rhs=ubf[:],
                                 start=True, stop=True)

                d2 = apool.tile([128, OCT_H], F32, tag="d2")
                if S[k] == 0:
                    nc.vector.memset(d2[:], 1000.0)

                for si in range(S[k]):
                    m = wpool.tile([128, OCT_H], F32, tag="m")
                    nc.scalar.activation(m[:], yr[:], AF.Abs,
                                         bias=coef[:, col + 3:col + 4],
                                         scale=coef[:, col + 2:col + 3])
                    c2 = wpool.tile([128, OCT_H], F32, tag="c2")
                    if si % 3 == 2:
                        # DVE path for (scC*y + bC)^2 (TS 2x-mode + TT self-mult)
                        u = wpool.tile([128, OCT_H], F32, tag="u")
                        nc.vector.tensor_scalar(
                            out=u[:], in0=yr[:], scalar1=coef[:, col + 0:col + 1],
                            scalar2=coef[:, col + 1:col + 2], op0=OP.mult, op1=OP.add)
                        nc.vector.tensor_tensor(out=c2[:], in0=u[:], in1=u[:],
                                                op=OP.mult)
                    else:
                        nc.scalar.activation(c2[:], yr[:], AF.Square,
                                             bias=coef[:, col + 1:col + 2],
                                             scale=coef[:, col + 0:col + 1])
                    if si == 0:
                        # first candidate initializes d2 directly
                        nc.vector.scalar_tensor_tensor(
                            out=d2[:], in0=m[:], scalar=coef[:, col + 4:col + 5],
                            in1=c2[:], op0=OP.subtract, op1=OP.max)
                    else:
                        cand = wpool.tile([128, OCT_H], F32, tag="cand")
                        nc.vector.scalar_tensor_tensor(
                            out=cand[:], in0=m[:], scalar=coef[:, col + 4:col + 5],
                            in1=c2[:], op0=OP.subtract, op1=OP.max)
                        nc.vector.tensor_tensor(out=d2[:], in0=d2[:], in1=cand[:],
                                                op=OP.min)
                    col += 5

                for vi in range(V[k]):
                    sqv = wpool.tile([128, OCT_H], F32, tag="sqv")
                    nc.scalar.activation(sqv[:], yr[:], AF.Square,
                                         bias=coef[:, col + 0:col + 1], scale=1.0)
                    nc.vector.scalar_tensor_tensor(
                        out=d2[:], in0=sqv[:], scalar=coef[:, col + 1:col + 2],
                        in1=d2[:], op0=OP.add, op1=OP.min)
                    col += 2

                # finals: sd2 halves of a phase pair share one tile, one sigmoid
                if k % 2 == 0:
                    sd2p = apool.tile([128, 2 * OCT_H], F32, tag="sd2p")
                nc.vector.scalar_tensor_tensor(
                    out=sd2p[:, (k % 2) * OCT_H:(k % 2 + 1) * OCT_H],
                    in0=par[:], scalar=coef[:, col + 0:col + 1],
                    in1=d2[:], op0=OP.subtract, op1=OP.mult)
                if k % 2 == 1:
                    val = wpool.tile([128, 2 * OCT_H], F32, tag="val")
                    nc.scalar.activation(val[:], sd2p[:], AF.Sigmoid,
                                         bias=0.0, scale=2.0)
                    nc.sync.dma_start(out=out_dram[k - 1], in_=val[:, 0:OCT_H])
                    nc.sync.dma_start(out=out_dram[k], in_=val[:, OCT_H:])
                col += 1

    nc.finalize()
    return nc


# ---------------------------------------------------------------------------
# entry point
# ---------------------------------------------------------------------------

def kernel(polygon):
    global LAST_RESULTS
    from concourse.bass_utils import run_bass_kernel_spmd

    in_maps, core_octs, S, V, ncol = _host_prep(polygon)
    nc = _build_program(S, V, ncol)
    trace = bool(int(os.environ.get("KERNEL_TRACE", "0")))
    res = run_bass_kernel_spmd(nc, in_maps, list(range(NCORES)), trace=trace)
    LAST_RESULTS = res

    full = np.zeros((W, H), dtype=np.float32)   # x-major
    for c in range(NCORES):
        o = res.results[c]["out"]
        for k in range(NOCT):
            s, oq = core_octs[c][k]
            full[s * 128:(s + 1) * 128, oq * OCT_H:(oq + 1) * OCT_H] = o[k]
    return np.ascontiguousarray(full.T)



# revision 7
# speedup vs baseline: 1.2633x; 1.2633x over previous
"""TRN2 Bass kernel for soft 2D polygon rasterization (1024x1024, 64-edge polygon).

Strategy (one SPMD program on 8 cores, per-core behavior fully data-driven):
  - Layout: x (columns) on partitions, y (rows) on the free axis; 64 tiles of
    [128 cols x 128 rows]; each core processes 8, assigned by a host-side
    load-balancing local search minimizing the padded per-phase slot maxima.
  - Inside/outside parity: per-column signed crossing histogram (exact f32
    replication of the reference crossing test); prefix sum along y via one
    fp16 matmul per tile against a triangular 0/1 matrix (all histogram values
    are fp16-exact small ints plus +-1024 band-mask steps; the base parity of
    each 128-row band is folded into histogram row 0).
  - Distance: only pixels within ~5.5 px of the boundary need true distance;
    the host culls per (column, tile) and PACKS a different edge into each
    partition lane of a slot, so a tile's slot count is the max per-column
    count, not the union count. Per edge slot:
      * one DVE tensor_scalar (fp16, 4x mode) computes w~ = s~*j + b~, an
        along-edge coordinate scaled by 2*sqrt(BIG)/L so the endpoint-slab
        test becomes w~^2 > BIG uniformly across packed lanes,
      * one fused custom DVE op (registered at import into concourse's
        custom-DVE table): d2 = min(d2, max(w~^2 - BIG, u^2)) where
        u = scC*j + bC is generated internally by an ADD-scan, so each edge
        costs two vector instructions total. fp16 overflow of w~ saturates to
        +inf which max/min handle correctly (candidate drops out).
    A tunable fraction of slots instead runs on ACT (two fused Square ops)
    with the cand/min on GpSimd, to balance the three engines.
  - Vertices (segment endpoints) cover the beyond-slab region exactly:
    one fused custom DVE op d2 = min(d2, (j+kv)^2 + q[x]) per vertex slot.
  - Finals: sd2 = (par - sc)*d2 (DVE, PSUM), sigmoid(-2*sd2) on ACT for a
    [128,256] pair of tiles; tiles with no nearby edges skip the distance
    pipeline entirely: val = sigmoid(4000*par + bias) straight from PSUM.
  - Output bf16 (absmax tolerance is 2e-2; bf16 rounds by <=2^-9 rel),
    converted to f32 host-side. Inputs: one coef DMA + one histogram DMA.
"""
import os
import numpy as np

W = H = 1024
NCORES = 8
OCT_H = 128          # tile rows
NOCT = 8             # tiles per core
R_KEEP = 2.5         # cull radius: dropping features >2.5px away errs <= e^-12.5
BIG = 4.0e5          # slab test scale: w~ = (2*sqrt(BIG)/L)*(w - L/2), test w~^2>BIG
SQBIG = float(np.sqrt(BIG))
BANDK = 1024.0       # band-mask parity offset (fp16-exact with small ints)
ACT_EDGE_FRAC = 0.0  # fraction of edge slots routed to ACT+Pool
ACT_VERT_FRAC = 0.0  # fraction of vertex slots routed to ACT+Pool

LAST_RESULTS = None  # BassKernelResults of the most recent run (for test harness)

_OPS_REGISTERED = {}


# ---------------------------------------------------------------------------
# custom DVE ops (registered into concourse's table at build time)
# ---------------------------------------------------------------------------

def _register_custom_ops():
    """Register the fused candidate ops in concourse's custom-DVE registry.

    Returns dict of DveOp handles. Idempotent per process.
    """
    global _OPS_REGISTERED
    if _OPS_REGISTERED:
        return _OPS_REGISTERED
    from concourse import dve_ops
    from concourse.dve_spec import (
        Spec, Src0, Src1, C0, C1, C2, One, sq, maxx, minn, scan, lower, AluOp,
    )
    from concourse.dve_uop import DveOpSpec
    from concourse.dve_table_gen import dve_ver_for

    ver = dve_ver_for("TRN2")

    def _jgrid(in0):
        j = np.arange(in0.shape[-1], dtype=np.float32)[None, :]
        return j

    # u_k = C1 + (k+1)*C0  (scan of constant C0 seeded with C1)
    def ref_edge_min(in0, in1, s0, s1, imm2):
        j = _jgrid(in0)
        u = (s1 + (j + 1.0) * s0).astype(np.float32)
        cand = np.maximum(in0.astype(np.float32) ** 2 - imm2, u * u)
        return np.minimum(in1.astype(np.float32), cand).astype(np.float32)

    def ref_edge0(in0, in1, s0, s1, imm2):
        j = _jgrid(in0)
        u = (s1 + (j + 1.0) * s0).astype(np.float32)
        return np.maximum(in0.astype(np.float32) ** 2 - imm2, u * u).astype(np.float32)

    def ref_vert_min(in0, in1, s0, s1, imm2):
        j = _jgrid(in0)
        t = s0 + (j + 1.0)
        return np.minimum(in0.astype(np.float32), t * t + s1).astype(np.float32)

    specs = {
        # d2 = min(d2, max(w~^2 - BIG, u^2)); in0=w~, in1=d2, s0=scC, s1=bC-scC, imm2=BIG
        "POLY_EDGE_MIN": (
            Spec(body=minn(Src1, maxx(sq(Src0) - C2,
                                      sq(scan(AluOp.ADD, C0, init=C1)))),
                 reference=ref_edge_min), True),
        # d2 = max(w~^2 - BIG, u^2)  (first slot, initializes d2)
        "POLY_EDGE0": (
            Spec(body=maxx(sq(Src0) - C2, sq(scan(AluOp.ADD, C0, init=C1))),
                 reference=ref_edge0), False),
        # d2 = min(d2, (j + kv)^2 + q); in0=d2 (in place), s0=kv-1, s1=q
        "POLY_VERT_MIN": (
            Spec(body=minn(Src0, sq(scan(AluOp.ADD, One, init=C0)) + C1),
                 reference=ref_vert_min), False),
    }
    row = max(dve_ops._SUB_OPCODE_FOR_NAME.values(), default=0)
    for name, (spec, rd1) in specs.items():
        if name in dve_ops._SUB_OPCODE_FOR_NAME:
            op = next(o for o in dve_ops.OPS if o.name == name)
            _OPS_REGISTERED[name] = op
            continue
        row += 1
        assert row < 0x20, "custom-DVE opcode rows exhausted"
        dve_ops._SUB_OPCODE_FOR_NAME[name] = row
        tmp = DveOpSpec(name=name, opcode=row, uops=lower(spec, ver=ver), rd1_en=rd1)
        op = dve_ops.DveOp(name=name, spec=spec, subdim=False,
                           uops_sha={ver: tmp.sha(ver)})
        dve_ops.OPS.append(op)
        _OPS_REGISTERED[name] = op
    return _OPS_REGISTERED


# ---------------------------------------------------------------------------
# host-side geometry prep
# ---------------------------------------------------------------------------

def _seg_vseg_dist2(ax, ay, bx, by, x, y0, y1):
    """Squared distance from segment (a,b) to the vertical segment
    {x} x [y0,y1], vectorized over columns x (1-D array)."""
    # sample the edge densely enough: use exact calc via clamped projections
    # distance between two segments = min over (closest point pairs);
    # compute via: min(dist(a-b, each endpoint of vseg), dist(vseg, each
    # endpoint of a-b), 0 if intersect)
    x = np.asarray(x, dtype=np.float64)
    abx, aby = bx - ax, by - ay
    L2 = abx * abx + aby * aby

    def pt_seg(px, py, sx0, sy0, sx1, sy1):
        dx, dy = sx1 - sx0, sy1 - sy0
        ll = dx * dx + dy * dy
        if np.isscalar(ll) and ll < 1e-18:
            return (px - sx0) ** 2 + (py - sy0) ** 2
        t = np.clip(((px - sx0) * dx + (py - sy0) * dy) / np.maximum(ll, 1e-18), 0, 1)
        return (sx0 + t * dx - px) ** 2 + (sy0 + t * dy - py) ** 2

    # edge endpoints to vseg
    d2 = np.minimum(pt_seg(ax, ay, x, y0, x, y1), pt_seg(bx, by, x, y0, x, y1))
    # vseg endpoints to edge
    d2 = np.minimum(d2, pt_seg(x, y0, ax, ay, bx, by))
    d2 = np.minimum(d2, pt_seg(x, y1, ax, ay, bx, by))
    # intersection test: edge crosses the vertical line within [y0,y1]
    with np.errstate(divide='ignore', invalid='ignore'):
        tx = np.where(abs(abx) > 1e-18, (x - ax) / (abx if abx != 0 else 1.0), -1.0)
    yx = ay + tx * aby
    hit = (tx >= 0) & (tx <= 1) & (yx >= y0) & (yx <= y1)
    return np.where(hit, 0.0, d2)


def _host_prep(polygon):
    poly = np.asarray(polygon, dtype=np.float32)
    E = poly.shape[0]
    a = poly
    b = np.roll(poly, -1, axis=0)
    ab = b - a

    # bbox band (exact f32 replication of the reference)
    x_lo = np.float32(np.floor(poly[:, 0].min()))
    y_lo = np.float32(np.floor(poly[:, 1].min()))
    x_hi = np.float32(np.floor(poly[:, 0].max()) + np.float32(1.0))
    y_hi = np.float32(np.floor(poly[:, 1].max()) + np.float32(1.0))
    thr = np.float32(30.0)
    xband_lo = x_lo - thr
    xband_hi = x_hi + thr
    yband_lo = y_lo - thr
    yband_hi = y_hi + thr

    # ---- signed crossing histogram (exact f32 semantics) ----
    PX = np.arange(W, dtype=np.float32)[None, :]
    a0 = a[:, 0:1]; a1 = a[:, 1:2]; b0 = b[:, 0:1]
    ab0 = ab[:, 0:1]; ab1 = ab[:, 1:2]
    crosses = (a0 <= PX) != (b0 <= PX)                       # [E, W]
    safe_dx = np.where(ab0 == np.float32(0.0), np.float32(1.0), ab0)
    with np.errstate(over='ignore', invalid='ignore'):
        yint = a1 + (PX - a0) * ab1 / safe_dx                # [E, W] f32
    bins = np.where(crosses, np.ceil(yint.astype(np.float64)), np.inf)
    bins = np.where(bins < 0, 0.0, bins)                     # clamp below
    bins = np.where(bins > H - 1, np.inf, bins)              # >1023 never hits
    srt = np.sort(bins, axis=0)                              # per column asc
    sign = np.where((np.arange(E)[:, None] % 2) == 0, 1.0, -1.0)
    hist = np.zeros((H, W), dtype=np.float64)
    valid = np.isfinite(srt)
    kk = srt[valid].astype(np.int64)
    jj = np.broadcast_to(np.arange(W)[None, :], (E, W))[valid]
    np.add.at(hist, (kk, jj), np.broadcast_to(sign, (E, W))[valid])
    csum = np.cumsum(hist, axis=0)                           # parity at row i

    # first/last in-band rows
    r_lo = int(np.ceil(float(yband_lo)))
    r_hi = int(np.floor(float(yband_hi)))
    xmask = ~((np.arange(W) >= float(xband_lo)) & (np.arange(W) <= float(xband_hi)))

    # ---- per-(column, tile) packed candidate lists (f64 geometry) ----
    A = a.astype(np.float64); B = b.astype(np.float64); AB = B - A
    L2 = AB[:, 0] ** 2 + AB[:, 1] ** 2
    Lc = np.sqrt(np.maximum(L2, 1e-12))
    good = L2 > 1e-9

    # col_edges[s][o][xl] = list of edge ids; col_verts similar
    col_edges = [[[[] for _ in range(128)] for _ in range(NOCT)] for _ in range(8)]
    col_verts = [[[[] for _ in range(128)] for _ in range(NOCT)] for _ in range(8)]
    for e in range(E):
        ax, ay = A[e]; bx, by = B[e]
        if good[e]:
            xlo = int(np.floor(min(ax, bx) - R_KEEP))
            xhi = int(np.ceil(max(ax, bx) + R_KEEP))
            xlo = max(xlo, 0); xhi = min(xhi, W - 1)
            if xlo <= xhi:
                xs = np.arange(xlo, xhi + 1)
                ylo_e = min(ay, by) - R_KEEP
                yhi_e = max(ay, by) + R_KEEP
                o0 = max(0, int(np.floor(ylo_e)) // OCT_H)
                o1 = min(NOCT - 1, int(np.ceil(yhi_e)) // OCT_H)
                for o in range(o0, o1 + 1):
                    y0, y1 = o * OCT_H, o * OCT_H + OCT_H - 1
                    d2cols = _seg_vseg_dist2(ax, ay, bx, by, xs, y0, y1)
                    for x, dd in zip(xs, d2cols):
                        if dd <= R_KEEP * R_KEEP:
                            col_edges[x // 128][o][x % 128].append(e)
        # vertex a of edge e
        xlo = max(0, int(np.floor(ax - R_KEEP)))
        xhi = min(W - 1, int(np.ceil(ax + R_KEEP)))
        ylo = ay - R_KEEP; yhi = ay + R_KEEP
        o0 = max(0, int(np.floor(ylo)) // OCT_H)
        o1 = min(NOCT - 1, int(np.ceil(yhi)) // OCT_H)
        for o in range(o0, o1 + 1):
            for x in range(xlo, xhi + 1):
                col_verts[x // 128][o][x % 128].append(e)

    nS = np.zeros((8, NOCT), dtype=int)
    nV = np.zeros((8, NOCT), dtype=int)
    for s in range(8):
        for o in range(NOCT):
            nS[s, o] = max(len(c) for c in col_edges[s][o])
            nV[s, o] = max(len(c) for c in col_verts[s][o])
            if nV[s, o] > 0 and nS[s, o] == 0:
                nS[s, o] = 1  # force an edge slot so d2 gets initialized

    # ---- octant -> (core, phase) assignment (balance padded maxima) ----
    octs = [(s, o) for s in range(8) for o in range(NOCT)]
    cE, cV = 2.0, 1.0
    cost = {so: cE * nS[so] + cV * nV[so] for so in octs}

    def padded_cost(assign):
        tot = 0.0
        ranked = [sorted(aa, key=lambda so: -cost[so]) for aa in assign]
        for k in range(NOCT):
            tot += cE * max(nS[r[k]] for r in ranked)
            tot += cV * max(nV[r[k]] for r in ranked)
        return tot

    order = sorted(octs, key=lambda so: -cost[so])
    core_load = [0.0] * NCORES
    assign = [[] for _ in range(NCORES)]
    for so in order:
        cands = [c for c in range(NCORES) if len(assign[c]) < NOCT]
        c = min(cands, key=lambda c: core_load[c])
        assign[c].append(so)
        core_load[c] += cost[so]
    best = padded_cost(assign)
    rng = np.random.default_rng(0)
    for _ in range(6000):
        c1, c2 = rng.integers(0, NCORES, 2)
        if c1 == c2:
            continue
        i1, i2 = rng.integers(0, NOCT, 2)
        assign[c1][i1], assign[c2][i2] = assign[c2][i2], assign[c1][i1]
        newc = padded_cost(assign)
        if newc <= best:
            best = newc
        else:
            assign[c1][i1], assign[c2][i2] = assign[c2][i2], assign[c1][i1]
    core_octs = [sorted(aa, key=lambda so: -cost[so]) for aa in assign]

    S = [max(nS[core_octs[c][k]] for c in range(NCORES)) for k in range(NOCT)]
    V = [max(nV[core_octs[c][k]] for c in range(NCORES)) for k in range(NOCT)]

    # phase k is an "edge phase" iff S[k] > 0 (sorted desc, so a prefix);
    # pad edge-phase count to even for output pairing
    NE = sum(1 for k in range(NOCT) if S[k] > 0)
    if NE % 2 == 1:
        if NE < NOCT:
            S[NE] = max(S[NE], 1)
            NE += 1
        else:
            NE -= 1  # cannot happen for sane polygons (kept for safety)

    # route split per phase: ACT slots are the TAIL slot indices
    Sa = [min(int(round(S[k] * ACT_EDGE_FRAC)), max(S[k] - 1, 0)) for k in range(NOCT)]
    Sd = [S[k] - Sa[k] for k in range(NOCT)]
    Va = [int(round(V[k] * ACT_VERT_FRAC)) for k in range(NOCT)]
    Vd = [V[k] - Va[k] for k in range(NOCT)]

    # ---- per-core input tensors ----
    # coef layout per edge phase k: Sd[k]*(st, bt, scC, bCs) + Sa[k]*(st, bt,
    # scC, bC) + Vd[k]*(kv1, q) + Va[k]*(kv, q); then per-phase: sc, xbias.
    # ACT-route w~ is scaled by an extra 1/4 so m2 = w~^2 stays below fp16 max
    # inside the slab (threshold BIG/16 = 25000).
    base_w = sum(4 * S[k] + 2 * V[k] for k in range(NE))
    ncol = base_w + 2 * NOCT
    in_maps = []
    for c in range(NCORES):
        coef = np.zeros((128, ncol), dtype=np.float32)
        histc = np.zeros((NOCT, OCT_H, 128), dtype=np.float64)
        col = 0
        for k in range(NOCT):
            s, o = core_octs[c][k]
            i0 = o * OCT_H
            # --- fp16 histogram block with band + base folded in ---
            hloc = np.ascontiguousarray(hist[i0:i0 + OCT_H, s * 128:(s + 1) * 128])
            if i0 > 0:
                base = np.mod(csum[i0 - 1, s * 128:(s + 1) * 128], 2.0)
                hloc[0, :] += base
            # y-band: out-of-band rows get -BANDK (pushes sigmoid arg to -inf)
            rl = r_lo - i0          # first in-band local row
            rh1 = r_hi + 1 - i0     # first out-of-band local row above
            if rl > 0:              # rows [0, min(rl,128)) start masked
                hloc[0, :] -= BANDK
                if rl <= OCT_H - 1:
                    hloc[rl, :] += BANDK
            if rh1 <= 0:
                hloc[0, :] -= BANDK
            elif rh1 <= OCT_H - 1:
                hloc[rh1, :] -= BANDK
            histc[k] = hloc
            # per-phase global columns
            xm = xmask[s * 128:(s + 1) * 128]
            coef[:, base_w + 2 * k + 0] = (0.5 + BANDK * xm).astype(np.float32)
            coef[:, base_w + 2 * k + 1] = (-2000.0 - 8.0e6 * xm).astype(np.float32)

            if k >= NE:
                continue
            # --- packed per-column slot coefficients ---
            eg = col_edges[s][o]
            vt = col_verts[s][o]
            for si in range(S[k]):
                dve_route = si < Sd[k]
                down = 1.0 if dve_route else 0.25
                st_c = np.zeros(128, dtype=np.float64)
                bt_c = np.zeros(128, dtype=np.float64)
                sc_c = np.zeros(128, dtype=np.float64)
                bc_c = np.full(128, 60.0, dtype=np.float64)   # dummy: u=60
                for xl in range(128):
                    lst = eg[xl]
                    if si < len(lst):
                        e = lst[si]
                        x = s * 128 + xl
                        L = Lc[e]
                        sig = down * 2.0 * SQBIG / L
                        # w~ = sig*( (x-Ax)*ABx/L + (i0 + j - Ay)*ABy/L - L/2 )
                        st_c[xl] = sig * AB[e, 1] / L
                        bt_c[xl] = sig * ((x - A[e, 0]) * AB[e, 0] / L
                                          + (i0 - A[e, 1]) * AB[e, 1] / L - L / 2.0)
                        # u = (x-Ax)*ABy/L - (i0 + j - Ay)*ABx/L
                        sc_c[xl] = -AB[e, 0] / L
                        bc_c[xl] = ((x - A[e, 0]) * AB[e, 1] / L
                                    - (i0 - A[e, 1]) * AB[e, 0] / L)
                coef[:, col + 0] = st_c
                coef[:, col + 1] = bt_c
                coef[:, col + 2] = sc_c
                # DVE route uses the scan form u_j = (bC - scC) + (j+1)*scC
                coef[:, col + 3] = (bc_c - sc_c) if dve_route else bc_c
                col += 4
            for vi in range(V[k]):
                dve_route = vi < Vd[k]
                kv_c = np.full(128, 200.0, dtype=np.float64)  # dummy, no-op
                q_c = np.zeros(128, dtype=np.float64)
                for xl in range(128):
                    lst = vt[xl]
                    if vi < len(lst):
                        e = lst[vi]
                        x = s * 128 + xl
                        kv_c[xl] = i0 - A[e, 1]
                        q_c[xl] = (x - A[e, 0]) ** 2
                coef[:, col + 0] = (kv_c - 1.0) if dve_route else kv_c
                coef[:, col + 1] = q_c
                col += 2
        hist16 = histc.astype(np.float16)
        assert np.all(hist16.astype(np.float64) == histc), "hist not fp16-exact"
        # DRAM layout [y_local, k*128 + x] so a single DMA fills SBUF
        in_maps.append({
            "coef": coef,
            "hist": np.ascontiguousarray(
                histc.transpose(1, 0, 2).reshape(OCT_H, NOCT * 128)
            ).astype(np.float16),
        })

    meta = dict(S=S, V=V, Sd=Sd, Sa=Sa, Vd=Vd, Va=Va, NE=NE, ncol=ncol,
                core_octs=core_octs)
    return in_maps, meta


# ---------------------------------------------------------------------------
# device program
# ---------------------------------------------------------------------------

def _build_program(meta):
    import concourse.bacc as bacc
    import concourse.mybir as mybir
    from concourse.tile import TileContext

    ops = _register_custom_ops()
    EDGE_MIN = ops["POLY_EDGE_MIN"]
    EDGE0 = ops["POLY_EDGE0"]
    VERT_MIN = ops["POLY_VERT_MIN"]

    F32 = mybir.dt.float32
    F16 = mybir.dt.float16
    BF16 = mybir.dt.bfloat16
    I32 = mybir.dt.int32
    AF = mybir.ActivationFunctionType
    OP = mybir.AluOpType

    S, V = meta["S"], meta["V"]
    Sd, Sa = meta["Sd"], meta["Sa"]
    Vd, Va = meta["Vd"], meta["Va"]
    NE, ncol = meta["NE"], meta["ncol"]
    base_w = sum(4 * S[k] + 2 * V[k] for k in range(NE))

    nc = bacc.Bacc()
    coef_in = nc.declare_dram_parameter("coef", [128, ncol], F32, isOutput=False)
    hist_in = nc.declare_dram_parameter("hist", [OCT_H, NOCT * 128], F16,
                                        isOutput=False)
    out_dram = nc.declare_dram_parameter("out", [NOCT, 128, OCT_H], BF16,
                                         isOutput=True)
    BIGA = BIG / 16.0  # ACT-route slab threshold (w~ pre-scaled by 1/4)

    with TileContext(nc) as tc:
        with tc.tile_pool(name="const", bufs=1) as cpool, \
             tc.tile_pool(name="work", bufs=3) as wpool, \
             tc.tile_pool(name="acc", bufs=3) as apool, \
             tc.tile_pool(name="ps", bufs=4, space="PSUM") as psum:

            coef = cpool.tile([128, ncol], F32)
            nc.sync.dma_start(out=coef[:], in_=coef_in[:])
            # hist DRAM is [y_local, k*128 + x]: one DMA, partitions = y_local
            hall = cpool.tile([128, NOCT * 128], F16)
            nc.sync.dma_start(out=hall[:], in_=hist_in[:])

            # warmup: trigger ACT table load while DMAs are in flight
            warm = cpool.tile([128, 1], F32)
            nc.vector.memset(warm[:], 0.0)
            nc.scalar.activation(warm[:], warm[:], AF.Sigmoid, bias=0.0, scale=1.0)

            # yr fp16 iota (row index within octant), ubf fp16 triangular
            yi = cpool.tile([128, OCT_H], I32)
            nc.gpsimd.iota(yi[:], pattern=[[1, OCT_H]], base=0, channel_multiplier=0)
            yr = cpool.tile([128, OCT_H], F16)
            nc.vector.tensor_copy(out=yr[:], in_=yi[:])
            ui = cpool.tile([128, OCT_H], I32)
            nc.gpsimd.iota(ui[:], pattern=[[1, OCT_H]], base=0, channel_multiplier=-1)
            ubf = cpool.tile([128, OCT_H], F16)
            nc.vector.tensor_scalar(out=ubf[:], in0=ui[:], scalar1=0, scalar2=None,
                                    op0=OP.is_ge)

            col = 0
            for k in range(NE):
                sc_col = coef[:, base_w + 2 * k + 0:base_w + 2 * k + 1]

                par = psum.tile([128, OCT_H], F32, tag="par")
                nc.tensor.matmul(par[:], lhsT=hall[:, k * OCT_H:(k + 1) * OCT_H],
                                 rhs=ubf[:], start=True, stop=True)

                if k % 2 == 0:
                    d2p = apool.tile([128, 2 * OCT_H], F16, tag="d2p")
                    sd2p = apool.tile([128, 2 * OCT_H], F32, tag="sd2p")
                d2 = d2p[:, (k % 2) * OCT_H:(k % 2 + 1) * OCT_H]

                # --- DVE-route edge slots ---
                for si in range(Sd[k]):
                    wt = wpool.tile([128, OCT_H], F16, tag=f"wt{si % 3}")
                    nc.vector.tensor_scalar(
                        out=wt[:], in0=yr[:],
                        scalar1=coef[:, col + 0:col + 1],
                        scalar2=coef[:, col + 1:col + 2],
                        op0=OP.mult, op1=OP.add)
                    if si == 0:
                        nc.vector._custom_dve(
                            EDGE0, out=d2, in0=wt[:],
                            s0=coef[:, col + 2:col + 3],
                            s1=coef[:, col + 3:col + 4], imm2=BIG)
                    else:
                        nc.vector._custom_dve(
                            EDGE_MIN, out=d2, in0=wt[:], in1=d2,
                            s0=coef[:, col + 2:col + 3],
                            s1=coef[:, col + 3:col + 4], imm2=BIG)
                    col += 4

                # --- DVE-route vertex slots (chain on d2) ---
                vcol = col + 4 * Sa[k]
                for vi in range(Vd[k]):
                    nc.vector._custom_dve(
                        VERT_MIN, out=d2, in0=d2,
                        s0=coef[:, vcol + 0:vcol + 1],
                        s1=coef[:, vcol + 1:vcol + 2])
                    vcol += 2

                # --- ACT-route edge slots (cand/min on gpsimd, SBUF only) ---
                have_b = Sa[k] > 0 or Va[k] > 0
                if have_b:
                    d2b = apool.tile([128, OCT_H], F16, tag="d2b")
                for si in range(Sa[k]):
                    m2 = wpool.tile([128, OCT_H], F16, tag="m2")
                    nc.scalar.activation(m2[:], yr[:], AF.Square,
                                         bias=coef[:, col + 1:col + 2],
                                         scale=coef[:, col + 0:col + 1])
                    c2 = wpool.tile([128, OCT_H], F16, tag="c2")
                    nc.scalar.activation(c2[:], yr[:], AF.Square,
                                         bias=coef[:, col + 3:col + 4],
                                         scale=coef[:, col + 2:col + 3])
                    if si == 0:
                        nc.gpsimd.scalar_tensor_tensor(
                            out=d2b[:], in0=m2[:], scalar=BIGA, in1=c2[:],
                            op0=OP.subtract, op1=OP.max)
                    else:
                        cnd = wpool.tile([128, OCT_H], F16, tag="cnd")
                        nc.gpsimd.scalar_tensor_tensor(
                            out=cnd[:], in0=m2[:], scalar=BIGA, in1=c2[:],
                            op0=OP.subtract, op1=OP.max)
                        nc.gpsimd.tensor_tensor(out=d2b[:], in0=d2b[:],
                                                in1=cnd[:], op=OP.min)
                    col += 4

                # --- ACT-route vertex slots ---
                vcol2 = col + 2 * Vd[k]
                for vi in range(Va[k]):
                    sqv = wpool.tile([128, OCT_H], F16, tag="sqv")
                    nc.scalar.activation(sqv[:], yr[:], AF.Square,
                                         bias=coef[:, vcol2 + 0:vcol2 + 1],
                                         scale=1.0)
                    if Sa[k] == 0 and vi == 0:
                        nc.gpsimd.tensor_scalar(
                            out=d2b[:], in0=sqv[:],
                            scalar1=coef[:, vcol2 + 1:vcol2 + 2], scalar2=None,
                            op0=OP.add)
                    else:
                        nc.gpsimd.scalar_tensor_tensor(
                            out=d2b[:], in0=sqv[:],
                            scalar=coef[:, vcol2 + 1:vcol2 + 2], in1=d2b[:],
                            op0=OP.add, op1=OP.min)
                    vcol2 += 2
                col += 2 * Vd[k] + 2 * Va[k]

                if have_b:
                    nc.vector.tensor_tensor(out=d2, in0=d2, in1=d2b[:], op=OP.min)

                # final: sd2 = (par - sc) * d2 ; val = sigmoid(-2*sd2)... note
                # sign: want sigmoid(+2*(par-sc)*d2) so use scale=+2
                nc.vector.scalar_tensor_tensor(
                    out=sd2p[:, (k % 2) * OCT_H:(k % 2 + 1) * OCT_H],
                    in0=par[:], scalar=sc_col, in1=d2,
                    op0=OP.subtract, op1=OP.mult)
                if k % 2 == 1:
                    val = wpool.tile([128, 2 * OCT_H], BF16, tag="val")
                    nc.scalar.activation(val[:], sd2p[:], AF.Sigmoid,
                                         bias=0.0, scale=2.0)
                    nc.sync.dma_start(out=out_dram[k - 1], in_=val[:, 0:OCT_H])
                    nc.sync.dma_start(out=out_dram[k], in_=val[:, OCT_H:])

            # --- parity-only phases: val = sigmoid(4000*par + xbias) ---
            for k in range(NE, NOCT):
                xb_col = coef[:, base_w + 2 * k + 1:base_w + 2 * k + 2]
                par = psum.tile([128, OCT_H], F32, tag="par")
                nc.tensor.matmul(par[:], lhsT=hall[:, k * OCT_H:(k + 1) * OCT_H],
                                 rhs=ubf[:], start=True, stop=True)
                if (k - NE) % 2 == 0:
                    valp = wpool.tile([128, 2 * OCT_H], BF16, tag="valp")
                half = (k - NE) % 2
                nc.scalar.activation(valp[:, half * OCT_H:(half + 1) * OCT_H],
                                     par[:], AF.Sigmoid,
                                     bias=xb_col, scale=4000.0)
                if half == 1 or k == NOCT - 1:
                    lo = k - half
                    nc.sync.dma_start(out=out_dram[lo], in_=valp[:, 0:OCT_H])
                    if half == 1:
                        nc.sync.dma_start(out=out_dram[k], in_=valp[:, OCT_H:])

    nc.finalize()
    return nc


# ---------------------------------------------------------------------------
# entry point
# ---------------------------------------------------------------------------

def kernel(polygon):
    global LAST_RESULTS
    from concourse.bass_utils import run_bass_kernel_spmd

    in_maps, meta = _host_prep(polygon)
    nc = _build_program(meta)
    trace = bool(int(os.environ.get("KERNEL_TRACE", "0")))
    res = run_bass_kernel_spmd(nc, in_maps, list(range(NCORES)), trace=trace)
    LAST_RESULTS = res

    core_octs = meta["core_octs"]
    full = np.zeros((W, H), dtype=np.float32)   # x-major
    for c in range(NCORES):
        o = np.asarray(res.results[c]["out"]).astype(np.float32)
        for k in range(NOCT):
            s, oq = core_octs[c][k]
            full[s * 128:(s + 1) * 128, oq * OCT_H:(oq + 1) * OCT_H] = o[k]
    return np.ascontiguousarray(full.T)


# revision 11
# speedup vs baseline: 1.4807x; 1.1720x over previous
"""TRN2 Bass kernel for soft 2D polygon rasterization (1024x1024, 64-edge polygon).

Strategy (one SPMD program on 8 cores, per-core behavior fully data-driven):
  - Layout: y (rows) on partitions, x (columns) on the free axis; 64 tiles of
    [128 rows x 128 cols]; each core processes 8, assigned by a host-side
    load-balancing local search minimizing padded per-phase slot costs.
  - Inside/outside parity for ALL 8 tiles comes from ONE pair of fp16 matmuls:
    parT[i, k*128+x] = sum_y hist_k[y, x] * U[y <= i] with the shared
    triangular U as the stationary operand. The histogram carries the
    reference's exact f32 crossing parities plus +-1024 offsets implementing
    the bbox+threshold band mask in both axes and the base parity of each
    128-row band (all values fp16-exact).
  - Distance: only pixels within ~5.5 px of the boundary need true distance;
    the host culls per (row, tile) and PACKS a different edge into each
    partition lane of a slot, so a tile's slot count is the max per-row count.
    Each slot's ops are sliced to the slot's x-window (union of its edges'
    x-reach). Per edge slot:
      * one tensor_scalar computes w~ = s~*x + b~ (along-edge coordinate,
        scaled 2*sqrt(BIG)/L so the endpoint-slab test is w~^2 > BIG),
      * one fused custom DVE op (registered into concourse's custom-DVE
        table): d2 = min(d2, max(w~^2 - BIG, u^2)), where u (perpendicular
        distance) is generated internally by an ADD-scan. fp16 overflow of
        w~ saturates to +inf, which max/min handle correctly.
    Vertices (segment endpoints) cover the beyond-slab region exactly with
    one fused custom op over a ~16-col window: d2 = min(d2, (x-Ax)^2 + q).
  - Finals: one STT sd2 = (parT - 0.5)*d2 over all 4 edge tiles [128,512],
    one sigmoid(2*sd2) [128,512]; parity-only tiles take a single
    sigmoid(4000*parT - 2000) [128,512] straight from PSUM. One output DMA
    of [128, 1024] bf16 (absmax tolerance 2e-2; bf16 rounds by <=2^-9 rel).
"""
import os
import numpy as np

W = H = 1024
NCORES = 8
OCT_H = 128          # tile rows
NOCT = 8             # tiles per core
R_KEEP = 2.5         # cull radius: dropping features >2.5px away errs <= e^-12.5
R_WIN = 7.0          # x-window margin around an edge's x-extent
BIG = 4.0e5          # slab test scale: w~ = (2*sqrt(BIG)/L)*(w - L/2)
SQBIG = float(np.sqrt(BIG))
BANDK = 1024.0       # band-mask parity offset (fp16-exact with small ints)
TS_ON_POOL = bool(int(os.environ.get("KERNEL_TS_POOL", "0")))
DUMMY_D2 = 3600.0

LAST_RESULTS = None  # BassKernelResults of the most recent run (for test harness)

_OPS_REGISTERED = {}


# ---------------------------------------------------------------------------
# custom DVE ops (registered into concourse's table at build time)
# ---------------------------------------------------------------------------

def _register_custom_ops():
    global _OPS_REGISTERED
    if _OPS_REGISTERED:
        return _OPS_REGISTERED
    from concourse import dve_ops
    from concourse.dve_spec import (
        Spec, Src0, Src1, C0, C1, C2, One, sq, maxx, minn, scan, lower, AluOp,
    )
    from concourse.dve_uop import DveOpSpec
    from concourse.dve_table_gen import dve_ver_for

    ver = dve_ver_for("TRN2")

    # u_k = C1 + (k+1)*C0  (scan of constant C0 seeded with C1)
    def ref_edge_min(in0, in1, s0, s1, imm2):
        k = np.arange(in0.shape[-1], dtype=np.float32)[None, :]
        u = (s1 + (k + 1.0) * s0).astype(np.float32)
        cand = np.maximum(in0.astype(np.float32) ** 2 - imm2, u * u)
        return np.minimum(in1.astype(np.float32), cand).astype(np.float32)

    def ref_vert_min(in0, in1, s0, s1, imm2):
        k = np.arange(in0.shape[-1], dtype=np.float32)[None, :]
        t = (s0 + (k + 1.0)).astype(np.float32)
        return np.minimum(in0.astype(np.float32), t * t + s1).astype(np.float32)

    specs = {
        # d2 = min(d2, max(w~^2 - BIG, u^2)); in0=w~, in1=d2 (=out), s0=scC,
        # s1=bC(scan-shifted), imm2=BIG
        "POLY_EDGE_MIN": (
            Spec(body=minn(Src1, maxx(sq(Src0) - C2,
                                      sq(scan(AluOp.ADD, C0, init=C1)))),
                 reference=ref_edge_min), True),
        # d2 = min(d2, (scan)^2 + q); in0=d2 (in place), s0=kv(scan-shifted),
        # s1=q
        "POLY_VERT_MIN": (
            Spec(body=minn(Src0, sq(scan(AluOp.ADD, One, init=C0)) + C1),
                 reference=ref_vert_min), False),
    }
    row = max(dve_ops._SUB_OPCODE_FOR_NAME.values(), default=0)
    for name, (spec, rd1) in specs.items():
        if name in dve_ops._SUB_OPCODE_FOR_NAME:
            _OPS_REGISTERED[name] = next(o for o in dve_ops.OPS if o.name == name)
            continue
        row += 1
        assert row < 0x20, "custom-DVE opcode rows exhausted"
        dve_ops._SUB_OPCODE_FOR_NAME[name] = row
        tmp = DveOpSpec(name=name, opcode=row, uops=lower(spec, ver=ver), rd1_en=rd1)
        op = dve_ops.DveOp(name=name, spec=spec, subdim=False,
                           uops_sha={ver: tmp.sha(ver)})
        dve_ops.OPS.append(op)
        _OPS_REGISTERED[name] = op
    return _OPS_REGISTERED


# ---------------------------------------------------------------------------
# host-side geometry prep
# ---------------------------------------------------------------------------

def _seg_hseg_d2(ax, ay, bx, by, x0, x1, y):
    """Squared distance from segment (a,b) to horizontal segment
    [x0,x1] x {y}, vectorized over y (1-D array)."""
    y = np.asarray(y, dtype=np.float64)

    def pt_seg(px, py, sx0, sy0, dx, dy):
        ll = dx * dx + dy * dy
        t = np.clip(((px - sx0) * dx + (py - sy0) * dy) / max(ll, 1e-18), 0, 1)
        return (sx0 + t * dx - px) ** 2 + (sy0 + t * dy - py) ** 2

    abx, aby = bx - ax, by - ay
    # endpoints of edge to hseg: clamp x into [x0,x1]
    d2 = (np.clip(ax, x0, x1) - ax) ** 2 + (y - ay) ** 2
    d2 = np.minimum(d2, (np.clip(bx, x0, x1) - bx) ** 2 + (y - by) ** 2)
    # endpoints of hseg to edge
    d2 = np.minimum(d2, pt_seg(x0, y, ax, ay, abx, aby))
    d2 = np.minimum(d2, pt_seg(x1, y, ax, ay, abx, aby))
    # crossing test: edge crosses the horizontal line at y within [x0,x1]
    if abs(aby) > 1e-18:
        t = (y - ay) / aby
        xc = ax + t * abx
        hit = (t >= 0) & (t <= 1) & (xc >= x0) & (xc <= x1)
        d2 = np.where(hit, 0.0, d2)
    return d2


def _host_prep(polygon):
    poly = np.asarray(polygon, dtype=np.float32)
    E = poly.shape[0]
    a = poly
    b = np.roll(poly, -1, axis=0)
    ab = b - a

    # bbox band (exact f32 replication of the reference)
    x_lo = np.float32(np.floor(poly[:, 0].min()))
    y_lo = np.float32(np.floor(poly[:, 1].min()))
    x_hi = np.float32(np.floor(poly[:, 0].max()) + np.float32(1.0))
    y_hi = np.float32(np.floor(poly[:, 1].max()) + np.float32(1.0))
    thr = np.float32(30.0)
    xband_lo = x_lo - thr
    xband_hi = x_hi + thr
    yband_lo = y_lo - thr
    yband_hi = y_hi + thr

    # ---- signed crossing histogram (exact f32 semantics) ----
    PX = np.arange(W, dtype=np.float32)[None, :]
    a0 = a[:, 0:1]; a1 = a[:, 1:2]; b0 = b[:, 0:1]
    ab0 = ab[:, 0:1]; ab1 = ab[:, 1:2]
    crosses = (a0 <= PX) != (b0 <= PX)                       # [E, W]
    safe_dx = np.where(ab0 == np.float32(0.0), np.float32(1.0), ab0)
    with np.errstate(over='ignore', invalid='ignore'):
        yint = a1 + (PX - a0) * ab1 / safe_dx                # [E, W] f32
    bins = np.where(crosses, np.ceil(yint.astype(np.float64)), np.inf)
    bins = np.where(bins < 0, 0.0, bins)
    bins = np.where(bins > H - 1, np.inf, bins)
    srt = np.sort(bins, axis=0)
    sign = np.where((np.arange(E)[:, None] % 2) == 0, 1.0, -1.0)
    hist = np.zeros((H, W), dtype=np.float64)
    valid = np.isfinite(srt)
    kk = srt[valid].astype(np.int64)
    jj = np.broadcast_to(np.arange(W)[None, :], (E, W))[valid]
    np.add.at(hist, (kk, jj), np.broadcast_to(sign, (E, W))[valid])
    csum = np.cumsum(hist, axis=0)

    r_lo = int(np.ceil(float(yband_lo)))
    r_hi = int(np.floor(float(yband_hi)))
    xmask = ~((np.arange(W) >= float(xband_lo)) & (np.arange(W) <= float(xband_hi)))

    # ---- per-(row, tile) packed candidate lists (f64 geometry) ----
    A = a.astype(np.float64); B = b.astype(np.float64); AB = B - A
    L2 = AB[:, 0] ** 2 + AB[:, 1] ** 2
    Lc = np.sqrt(np.maximum(L2, 1e-12))
    good = L2 > 1e-9

    # row_edges[s][o][j] = list of edge ids (sorted by x-center);
    # row_verts[s][o][j] = list of vertex (edge) ids
    row_edges = [[[[] for _ in range(128)] for _ in range(NOCT)] for _ in range(8)]
    row_verts = [[[[] for _ in range(128)] for _ in range(NOCT)] for _ in range(8)]
    for e in range(E):
        ax, ay = A[e]; bx, by = B[e]
        if good[e]:
            ylo = max(0, int(np.floor(min(ay, by) - R_KEEP)))
            yhi = min(H - 1, int(np.ceil(max(ay, by) + R_KEEP)))
            for s in range(8):
                x0, x1 = s * 128, s * 128 + 127
                if max(ax, bx) < x0 - R_KEEP or min(ax, bx) > x1 + R_KEEP:
                    continue
                ys = np.arange(ylo, yhi + 1)
                d2r = _seg_hseg_d2(ax, ay, bx, by, x0, x1, ys)
                for y, dd in zip(ys, d2r):
                    if dd <= R_KEEP * R_KEEP:
                        row_edges[s][(y // OCT_H)][y % OCT_H].append(e)
        # vertex a of edge e
        s0v = max(0, int(np.floor(ax - R_KEEP)) // 128)
        s1v = min(7, int(np.ceil(ax + R_KEEP)) // 128)
        ylo = max(0, int(np.floor(ay - R_KEEP)))
        yhi = min(H - 1, int(np.ceil(ay + R_KEEP)))
        for s in range(s0v, s1v + 1):
            for y in range(ylo, yhi + 1):
                row_verts[s][y // OCT_H][y % OCT_H].append(e)
    # sort each row's edge list by x-center so slots cluster in x
    xc = (A[:, 0] + B[:, 0]) / 2
    for s in range(8):
        for o in range(NOCT):
            for j in range(128):
                row_edges[s][o][j].sort(key=lambda e: xc[e])
                row_verts[s][o][j].sort(key=lambda e: A[e, 0])

    nS = np.zeros((8, NOCT), dtype=int)
    nV = np.zeros((8, NOCT), dtype=int)
    for s in range(8):
        for o in range(NOCT):
            nS[s, o] = max(len(r) for r in row_edges[s][o])
            nV[s, o] = max(len(r) for r in row_verts[s][o])
            if nV[s, o] > 0 and nS[s, o] == 0:
                nS[s, o] = 1

    # per-(tile, slot) x-windows (local cols, padded to mult of 4)
    def slot_windows(s, o):
        wins_e, wins_v = [], []
        for si in range(nS[s, o]):
            lo, hi = 128, 0
            for j in range(128):
                lst = row_edges[s][o][j]
                if si < len(lst):
                    e = lst[si]
                    lo = min(lo, min(A[e, 0], B[e, 0]) - R_WIN - s * 128)
                    hi = max(hi, max(A[e, 0], B[e, 0]) + R_WIN - s * 128)
            lo = int(max(0, np.floor(lo))); hi = int(min(127, np.ceil(hi)))
            wins_e.append((lo, hi + 1) if lo <= hi else (0, 4))
        for vi in range(nV[s, o]):
            lo, hi = 128, 0
            for j in range(128):
                lst = row_verts[s][o][j]
                if vi < len(lst):
                    e = lst[vi]
                    lo = min(lo, A[e, 0] - R_WIN - s * 128)
                    hi = max(hi, A[e, 0] + R_WIN - s * 128)
            lo = int(max(0, np.floor(lo))); hi = int(min(127, np.ceil(hi)))
            wins_v.append((lo, hi + 1) if lo <= hi else (0, 4))
        return wins_e, wins_v

    tile_wins = {(s, o): slot_windows(s, o) for s in range(8) for o in range(NOCT)}

    # ---- tile -> (core, phase) assignment (balance padded window costs) ----
    octs = [(s, o) for s in range(8) for o in range(NOCT)]
    CE_FIX, CV_FIX = 140.0, 140.0   # per-op fixed ns
    def tile_cost(so):
        we, wv = tile_wins[so]
        c = sum(2 * (hi - lo) * 2.1 + 3 * CE_FIX for lo, hi in we)
        c += sum((hi - lo) * 2.1 + CV_FIX for lo, hi in wv)
        return c
    cost = {so: tile_cost(so) for so in octs}

    order = sorted(octs, key=lambda so: -cost[so])
    core_load = [0.0] * NCORES
    assign = [[] for _ in range(NCORES)]
    for so in order:
        cands = [c for c in range(NCORES) if len(assign[c]) < NOCT]
        c = min(cands, key=lambda c: core_load[c])
        assign[c].append(so)
        core_load[c] += cost[so]

    def padded_cost(asg):
        ranked = [sorted(aa, key=lambda so: -cost[so]) for aa in asg]
        tot = 0.0
        for k in range(NOCT):
            tiles = [r[k] for r in ranked]
            smax = max(nS[t] for t in tiles)
            vmax = max(nV[t] for t in tiles)
            for si in range(smax):
                lo = min(tile_wins[t][0][si][0] for t in tiles
                         if si < len(tile_wins[t][0])) if smax else 0
                hi = max(tile_wins[t][0][si][1] for t in tiles
                         if si < len(tile_wins[t][0])) if smax else 0
                tot += 2 * (hi - lo) * 2.1 + 3 * CE_FIX
            for vi in range(vmax):
                lo = min(tile_wins[t][1][vi][0] for t in tiles
                         if vi < len(tile_wins[t][1])) if vmax else 0
                hi = max(tile_wins[t][1][vi][1] for t in tiles
                         if vi < len(tile_wins[t][1])) if vmax else 0
                tot += (hi - lo) * 2.1 + CV_FIX
        return tot

    best = padded_cost(assign)
    rng = np.random.default_rng(0)
    for _ in range(4000):
        c1, c2 = rng.integers(0, NCORES, 2)
        if c1 == c2:
            continue
        i1, i2 = rng.integers(0, NOCT, 2)
        assign[c1][i1], assign[c2][i2] = assign[c2][i2], assign[c1][i1]
        newc = padded_cost(assign)
        if newc <= best:
            best = newc
        else:
            assign[c1][i1], assign[c2][i2] = assign[c2][i2], assign[c1][i1]
    core_octs = [sorted(aa, key=lambda so: -cost[so]) for aa in assign]

    # padded per-phase slot counts + windows
    S = []; V = []; EWIN = []; VWIN = []
    for k in range(NOCT):
        tiles = [core_octs[c][k] for c in range(NCORES)]
        smax = int(max(nS[t] for t in tiles))
        vmax = int(max(nV[t] for t in tiles))
        ew = []
        for si in range(smax):
            lo = min((tile_wins[t][0][si][0] for t in tiles
                      if si < len(tile_wins[t][0])), default=0)
            hi = max((tile_wins[t][0][si][1] for t in tiles
                      if si < len(tile_wins[t][0])), default=4)
            ext = hi - lo
            ext = min(128, (ext + 3) // 4 * 4)
            lo = min(lo, 128 - ext)
            ew.append((lo, ext))
        vw = []
        for vi in range(vmax):
            lo = min((tile_wins[t][1][vi][0] for t in tiles
                      if vi < len(tile_wins[t][1])), default=0)
            hi = max((tile_wins[t][1][vi][1] for t in tiles
                      if vi < len(tile_wins[t][1])), default=4)
            ext = hi - lo
            ext = min(128, (ext + 3) // 4 * 4)
            lo = min(lo, 128 - ext)
            vw.append((lo, ext))
        S.append(smax); V.append(vmax); EWIN.append(ew); VWIN.append(vw)

    # edge phases = prefix with S[k] > 0 (cost-sorted so this is a prefix)
    NE = sum(1 for k in range(NOCT) if S[k] > 0)
    assert NE <= 4, f"more than 4 edge phases per core ({NE}); layout assumes <=4"
    NE_PAD = 4  # blocks 0..3 are edge-final tiles, 4..7 parity tiles

    # ---- per-core input tensors ----
    # coef layout per edge phase k: S[k]*(st, bt, scC, bCs) + V[k]*(kv, q);
    # final column: parity-sigmoid bias (-2000, per-partition const)
    ncol = sum(4 * S[k] + 2 * V[k] for k in range(NE)) + 1
    in_maps = []
    for c in range(NCORES):
        coef = np.zeros((128, max(ncol, 1)), dtype=np.float32)
        coef[:, ncol - 1] = -2000.0
        histc = np.zeros((NOCT, OCT_H, 128), dtype=np.float64)
        col = 0
        for k in range(NOCT):
            s, o = core_octs[c][k]
            i0 = o * OCT_H
            # --- fp16 histogram block with band + base + xmask folded in ---
            hloc = np.ascontiguousarray(hist[i0:i0 + OCT_H, s * 128:(s + 1) * 128])
            if i0 > 0:
                base = np.mod(csum[i0 - 1, s * 128:(s + 1) * 128], 2.0)
                hloc[0, :] += base
            xm = xmask[s * 128:(s + 1) * 128]
            rl = r_lo - i0          # first in-band local row
            rh1 = r_hi + 1 - i0     # first out-of-band local row above
            ymask0 = np.zeros(128)  # row-0 offset for y-band
            if rl > 0:
                ymask0 -= BANDK
            if rh1 <= 0:
                ymask0 -= BANDK
            # out-of-x-band columns: flat -BANDK, no y-steps
            hloc[0, :] += np.where(xm, -BANDK, ymask0)
            if 0 < rl <= OCT_H - 1:
                hloc[rl, :] += np.where(xm, 0.0, BANDK)
            if 0 < rh1 <= OCT_H - 1:
                hloc[rh1, :] += np.where(xm, 0.0, -BANDK)
            histc[k] = hloc

            if k >= NE:
                continue
            # --- packed per-row slot coefficients ---
            eg = row_edges[s][o]
            vt = row_verts[s][o]
            for si in range(S[k]):
                xoff, ext = EWIN[k][si]
                st_c = np.zeros(128, dtype=np.float64)
                bt_c = np.zeros(128, dtype=np.float64)
                sc_c = np.zeros(128, dtype=np.float64)
                bcs_c = np.full(128, 60.0, dtype=np.float64)   # dummy: u=60
                for j in range(128):
                    lst = eg[j]
                    if si < len(lst):
                        e = lst[si]
                        y = i0 + j
                        L = Lc[e]
                        sig = 2.0 * SQBIG / L
                        # w~(x) = sig*((x-Ax)*ABx/L + (y-Ay)*ABy/L - L/2)
                        st_c[j] = sig * AB[e, 0] / L
                        bt_c[j] = sig * ((s * 128 - A[e, 0]) * AB[e, 0] / L
                                         + (y - A[e, 1]) * AB[e, 1] / L - L / 2.0)
                        # u(x) = (x-Ax)*ABy/L - (y-Ay)*ABx/L ; x = s*128+xoff+k+?
                        scC = AB[e, 1] / L
                        bC = ((s * 128 - A[e, 0]) * AB[e, 1] / L
                              - (y - A[e, 1]) * AB[e, 0] / L)
                        sc_c[j] = scC
                        # scan gives u_k = init + (k+1)*scC at local idx k;
                        # want scC*(xoff+k) + bC -> init = bC + scC*(xoff-1)
                        bcs_c[j] = bC + scC * (xoff - 1)
                coef[:, col + 0] = st_c
                coef[:, col + 1] = bt_c
                coef[:, col + 2] = sc_c
                coef[:, col + 3] = bcs_c
                col += 4
            for vi in range(V[k]):
                xoff, ext = VWIN[k][vi]
                kv_c = np.full(128, 300.0, dtype=np.float64)  # dummy, no-op
                q_c = np.full(128, DUMMY_D2, dtype=np.float64)
                for j in range(128):
                    lst = vt[j]
                    if vi < len(lst):
                        e = lst[vi]
                        y = i0 + j
                        # scan value at k: kv + (k+1); want (s*128+xoff+k)-Ax
                        kv_c[j] = s * 128 + xoff - 1 - A[e, 0]
                        q_c[j] = (y - A[e, 1]) ** 2
                coef[:, col + 0] = kv_c
                coef[:, col + 1] = q_c
                col += 2
        hist16 = histc.astype(np.float16)
        assert np.all(hist16.astype(np.float64) == histc), "hist not fp16-exact"
        in_maps.append({
            "coef": coef,
            "hist": np.ascontiguousarray(
                histc.transpose(1, 0, 2).reshape(OCT_H, NOCT * 128)
            ).astype(np.float16),
        })

    meta = dict(S=S, V=V, EWIN=EWIN, VWIN=VWIN, NE=NE, ncol=ncol,
                core_octs=core_octs)
    return in_maps, meta


# ---------------------------------------------------------------------------
# numpy simulator of the device program (host-side debugging)
# ---------------------------------------------------------------------------

def _simulate(in_maps, meta):
    S, V = meta["S"], meta["V"]
    EWIN, VWIN = meta["EWIN"], meta["VWIN"]
    NE = meta["NE"]
    core_octs = meta["core_octs"]
    outs = []
    xr = np.arange(128, dtype=np.float64)[None, :]
    U = (np.arange(128)[:, None] <= np.arange(128)[None, :])  # U[y,i]
    for c in range(NCORES):
        coef = in_maps[c]["coef"].astype(np.float64)
        hall = in_maps[c]["hist"].astype(np.float64)  # [y, k*128+x]
        out = np.zeros((NOCT, 128, 128), dtype=np.float64)  # [k, i, x]
        # parity: parT[i, k*128+x] = sum_y hall[y, k*128+x] * U[y,i]
        parT = np.einsum('yc,yi->ic', hall, U)  # [i, NOCT*128]
        col = 0
        for k in range(NOCT):
            par = parT[:, k * 128:(k + 1) * 128]  # [i, x]
            if k >= NE:
                out[k] = 1.0 / (1.0 + np.exp(-np.clip(4000.0 * par - 2000.0,
                                                      -700, 700)))
                continue
            d2 = np.full((128, 128), DUMMY_D2)
            for si in range(S[k]):
                xoff, ext = EWIN[k][si]
                st = coef[:, col + 0:col + 1]; bt = coef[:, col + 1:col + 2]
                sc = coef[:, col + 2:col + 3]; bcs = coef[:, col + 3:col + 4]
                xl = xr[:, xoff:xoff + ext]
                wt = np.float16(st * xl + bt).astype(np.float64)  # fp16 stream
                kk = np.arange(ext, dtype=np.float64)[None, :]
                u = bcs + (kk + 1.0) * sc
                cand = np.maximum(wt * wt - BIG, u * u)
                d2[:, xoff:xoff + ext] = np.float16(
                    np.minimum(d2[:, xoff:xoff + ext], cand))
                col += 4
            for vi in range(V[k]):
                xoff, ext = VWIN[k][vi]
                kv = coef[:, col + 0:col + 1]; q = coef[:, col + 1:col + 2]
                kk = np.arange(ext, dtype=np.float64)[None, :]
                t = kv + (kk + 1.0)
                d2[:, xoff:xoff + ext] = np.float16(
                    np.minimum(d2[:, xoff:xoff + ext], t * t + q))
                col += 2
            sd2 = (par - 0.5) * d2
            out[k] = 1.0 / (1.0 + np.exp(-np.clip(2.0 * sd2, -700, 700)))
        outs.append(out.astype(np.float32))
    # assemble
    full = np.zeros((H, W), dtype=np.float32)
    for c in range(NCORES):
        for k in range(NOCT):
            s, o = core_octs[c][k]
            full[o * 128:(o + 1) * 128, s * 128:(s + 1) * 128] = outs[c][k]
    return full


# ---------------------------------------------------------------------------
# device program
# ---------------------------------------------------------------------------

def _build_program(meta):
    import concourse.bacc as bacc
    import concourse.mybir as mybir
    from concourse.tile import TileContext

    ops = _register_custom_ops()
    EDGE_MIN = ops["POLY_EDGE_MIN"]
    VERT_MIN = ops["POLY_VERT_MIN"]

    F32 = mybir.dt.float32
    F16 = mybir.dt.float16
    BF16 = mybir.dt.bfloat16
    I32 = mybir.dt.int32
    AF = mybir.ActivationFunctionType
    OP = mybir.AluOpType

    S, V = meta["S"], meta["V"]
    EWIN, VWIN = meta["EWIN"], meta["VWIN"]
    NE, ncol = meta["NE"], meta["ncol"]

    nc = bacc.Bacc()
    coef_in = nc.declare_dram_parameter("coef", [128, ncol], F32, isOutput=False)
    hist_in = nc.declare_dram_parameter("hist", [OCT_H, NOCT * 128], F16,
                                        isOutput=False)
    out_dram = nc.declare_dram_parameter("out", [128, NOCT * 128], BF16,
                                         isOutput=True)

    with TileContext(nc) as tc:
        with tc.tile_pool(name="const", bufs=1) as cpool, \
             tc.tile_pool(name="work", bufs=4) as wpool, \
             tc.tile_pool(name="ps", bufs=1, space="PSUM") as psum:

            coef = cpool.tile([128, ncol], F32)
            nc.sync.dma_start(out=coef[:], in_=coef_in[:])
            hall = cpool.tile([128, NOCT * 128], F16)
            nc.sync.dma_start(out=hall[:], in_=hist_in[:])

            # warmup: trigger ACT table load while DMAs are in flight
            warm = cpool.tile([128, 1], F32)
            nc.vector.memset(warm[:], 0.0)
            nc.scalar.activation(warm[:], warm[:], AF.Sigmoid, bias=0.0, scale=1.0)

            # xr fp16 iota (col index), ubf fp16 triangular U[y,i] = (i >= y)
            xi = cpool.tile([128, 128], I32)
            nc.gpsimd.iota(xi[:], pattern=[[1, 128]], base=0, channel_multiplier=0)
            xr = cpool.tile([128, 128], F16)
            nc.vector.tensor_copy(out=xr[:], in_=xi[:])
            ui = cpool.tile([128, 128], I32)
            nc.gpsimd.iota(ui[:], pattern=[[1, 128]], base=0, channel_multiplier=-1)
            ubf = cpool.tile([128, 128], F16)
            nc.vector.tensor_scalar(out=ubf[:], in0=ui[:], scalar1=0, scalar2=None,
                                    op0=OP.is_ge)

            # one matmul pair: parT[i, k*128+x] for all 8 tiles
            parT = psum.tile([128, NOCT * 128], F32)
            nc.tensor.matmul(parT[:, 0:512], lhsT=ubf[:], rhs=hall[:, 0:512],
                             start=True, stop=True)
            nc.tensor.matmul(parT[:, 512:1024], lhsT=ubf[:], rhs=hall[:, 512:1024],
                             start=True, stop=True)

            # d2 for the 4 edge tiles
            d2q = wpool.tile([128, 4 * 128], F16, tag="d2q")
            nc.vector.memset(d2q[:], DUMMY_D2)

            ts_eng = nc.gpsimd if TS_ON_POOL else nc.vector
            col = 0
            for k in range(NE):
                d2 = d2q[:, k * 128:(k + 1) * 128]
                for si in range(S[k]):
                    xoff, ext = EWIN[k][si]
                    wt = wpool.tile([128, 128], F16, tag=f"wt{si % 3}")
                    ts_eng.tensor_scalar(
                        out=wt[:, 0:ext], in0=xr[:, xoff:xoff + ext],
                        scalar1=coef[:, col + 0:col + 1],
                        scalar2=coef[:, col + 1:col + 2],
                        op0=OP.mult, op1=OP.add)
                    nc.vector._custom_dve(
                        EDGE_MIN, out=d2[:, xoff:xoff + ext], in0=wt[:, 0:ext],
                        in1=d2[:, xoff:xoff + ext],
                        s0=coef[:, col + 2:col + 3],
                        s1=coef[:, col + 3:col + 4], imm2=BIG)
                    col += 4
                for vi in range(V[k]):
                    xoff, ext = VWIN[k][vi]
                    nc.vector._custom_dve(
                        VERT_MIN, out=d2[:, xoff:xoff + ext],
                        in0=d2[:, xoff:xoff + ext],
                        s0=coef[:, col + 0:col + 1],
                        s1=coef[:, col + 1:col + 2])
                    col += 2

            # finals: sd2 = (parT - 0.5) * d2 over all 4 edge tiles at once
            sd2q = wpool.tile([128, 4 * 128], F32, tag="sd2q")
            nc.vector.scalar_tensor_tensor(
                out=sd2q[:], in0=parT[:, 0:512], scalar=0.5, in1=d2q[:],
                op0=OP.subtract, op1=OP.mult)
            val = wpool.tile([128, NOCT * 128], BF16, tag="val")
            nc.scalar.activation(val[:, 0:512], sd2q[:], AF.Sigmoid,
                                 bias=0.0, scale=2.0)
            # parity-only tiles: val = sigmoid(4000*parT - 2000)
            nc.scalar.activation(val[:, 512:1024], parT[:, 512:1024], AF.Sigmoid,
                                 bias=coef[:, ncol - 1:ncol], scale=4000.0)
            nc.sync.dma_start(out=out_dram[:], in_=val[:])

    nc.finalize()
    return nc


# ---------------------------------------------------------------------------
# entry point
# ---------------------------------------------------------------------------

def kernel(polygon):
    global LAST_RESULTS
    from concourse.bass_utils import run_bass_kernel_spmd

    in_maps, meta = _host_prep(polygon)
    nc = _build_program(meta)
    trace = bool(int(os.environ.get("KERNEL_TRACE", "0")))
    res = run_bass_kernel_spmd(nc, in_maps, list(range(NCORES)), trace=trace)
    LAST_RESULTS = res

    core_octs = meta["core_octs"]
    full = np.zeros((H, W), dtype=np.float32)
    for c in range(NCORES):
        o = np.asarray(res.results[c]["out"]).astype(np.float32)  # [128, 8*128]
        for k in range(NOCT):
            s, oq = core_octs[c][k]
            full[oq * 128:(oq + 1) * 128, s * 128:(s + 1) * 128] = \
                o[:, k * 128:(k + 1) * 128]
    return np.ascontiguousarray(full)
